# revision 54
# baseline (speedup 1.0000x reference)
"""Distributed attention block on 8 TRN2 NeuronCores.

Reference math (torch Linear convention, no 1/sqrt(d) scale):
    q = x @ Wq.T + bq ; k = x @ Wk.T + bk ; v = x @ Wv.T + bv
    attn = softmax(q @ k.T, axis=-1)
    out = x + (attn @ v) @ Wo.T + bo

Output-projection folding: (attn @ v) @ Wo.T == attn @ (v @ Wo.T),
and v @ Wo.T = x @ (Wo Wv).T + Wo bv, so the kernel computes
u = x @ Wu.T + bu with host-premultiplied Wu = Wo @ Wv, bu = Wo @ bv,
gathers u instead of v, and finishes with out = x + attn @ u + bo.
This moves the entire output projection off the post-gather critical
tail and into phase A, where the PE would otherwise idle waiting for
the collective queue.

Sharding: rows of x (N=4096) split across 8 cores (512 rows each).
Each core computes its q tile; k and u are all-gathered in 2 chunks
each (k chunks first) so S starts when the first k chunk lands, and
attn@u consumes slices in the same chunk-major order.

Everything on-chip is computed in transposed layout ([C, n] feature
major) so biases are per-partition and QK^T is produced directly as
S.T (nj on partitions), which feeds attn@v without transposes. The
softmax denominators accumulate on the idle DVE (racc += expS tile),
partition-reduce + 1/x-broadcast on GpSimd/DVE, keeping the PE stream
pure matmul and all 8 PSUM banks free for the attn@v accumulators.

Key perf structure (HWDGE dispatch costs ~0.7us per dma_start on the
issuing engine's queue, so DMA count is minimized; the NRT collective
start barrier is a fixed ~21+35-65us from kernel start and each AG op
has a ~18.5us floor, so the chain is 4 ops of >=512KB):
  - weights / xT / consts each load as one or two big DMAs; W tiles
    live in a 2-slot pool (Wk dies after k-proj).
  - k staging is chunk-major in SBUF so each AG chunk stages with ONE
    contiguous DMA; u staging uses one rearranged-AP DMA per chunk.
  - kt gather readback on the sync queue per [128, .] half-tile; u
    readback as 1MB quads on the GpSimd SWDGE queue (so neither can
    head-of-line-block the other's dispatch queue).
  - ~100 junk matmuls on qT keep the PE clock gate (HAM) at 8/8
    through the idle window while AG_k0 is in flight.
  - residual uses the bf16 xT already in SBUF (no fp32 x copy).

Compute dtype bf16 (PSUM accumulation fp32). A global shift of -40 is
applied inside exp(): softmax is shift-invariant, the global logit max
~79 would otherwise ride close to fp32 overflow, and every row max is
>= 39.8 so denominators stay O(1).
"""

import numpy as np
import ml_dtypes

import concourse.bass as bass
import concourse.tile as tile
from concourse import bacc, bass_isa, mybir
from concourse.bass_utils import run_bass_kernel_spmd

N = 4096
C = 1024
R = 8            # cores
NL = N // R      # 512 rows per core
P = 128
CT = C // P      # 8 c tiles
KCHUNKS = [(0, 2), (2, 2)]   # (start nj-tile, n nj-tiles) per k AG chunk
NCH = len(KCHUNKS)
# v chunks: measured per-op floor is ~18.5us so fewer/bigger ops win
VCHUNKS = [(0, 2), (2, 2)]


def _t_of(j, m):
    """expS slice index for global nj-tile (rank j, tile m within rank),
    matching the k-chunk-major S production order."""
    for h, (st0, nt) in enumerate(KCHUNKS):
        if st0 <= m < st0 + nt:
            base = sum(R * n for _, n in KCHUNKS[:h])
            return base + j * nt + (m - st0)
    raise ValueError(m)
SHIFT = -40.0    # global logit shift inside exp

f32 = mybir.dt.float32
bf16 = mybir.dt.bfloat16
npbf = ml_dtypes.bfloat16

TRACE = False
_CACHE = {}


def _build():
    nc = bacc.Bacc("TRN2", target_bir_lowering=False, debug=False,
                   num_devices=R)

    # host-prepped layouts (see kernel() below)
    xT_d = nc.dram_tensor("xT", [P, CT * NL], bf16, kind="ExternalInput").ap()
    Wk_d = nc.dram_tensor("Wk2", [P, CT * C], bf16, kind="ExternalInput").ap()
    Wu_d = nc.dram_tensor("Wu2", [P, CT * C], bf16, kind="ExternalInput").ap()
    Wq_d = nc.dram_tensor("Wq2", [P, CT * C], bf16, kind="ExternalInput").ap()
    # [:, 0:8]=bqc  [:, 8:16]=bkc  [:, 16:24]=boc  [:, 24]=shift
    # [:, 32:160]=1.0f (ones row for the fp32 broadcast matmul)
    cst_d = nc.dram_tensor("cst", [P, 160], f32, kind="ExternalInput").ap()
    bones_d = nc.dram_tensor("bones", [P, NL], bf16, kind="ExternalInput").ap()
    bv_d = nc.dram_tensor("bvrow", [1, C], bf16, kind="ExternalInput").ap()
    ident_d = nc.dram_tensor("ident", [P, P], bf16, kind="ExternalInput").ap()
    outT_d = nc.dram_tensor("outT", [C, NL], bf16,
                            kind="ExternalOutput").ap()

    Exp = mybir.ActivationFunctionType.Exp
    Ident = mybir.ActivationFunctionType.Identity
    rg = [list(range(R))]

    with tile.TileContext(nc) as tc:
        with (
            tc.tile_pool(name="persist", bufs=1) as pp,
            tc.tile_pool(name="wpool", bufs=2) as wp,
            tc.tile_pool(name="ktp", bufs=10) as ktp,
            tc.tile_pool(name="vtp", bufs=4) as vtp,
            tc.tile_pool(name="dram", bufs=1, space="DRAM") as dp,
        ):
            # ---- front-loaded DMAs (sync queue; ~0.7us dispatch each) ----
            cst = pp.tile([P, 160], f32, tag="cst")
            nc.sync.dma_start(out=cst[:], in_=cst_d[:])
            bones = pp.tile([P, NL], bf16, tag="bones")
            nc.sync.dma_start(out=bones[:], in_=bones_d[:])
            bv = pp.tile([1, C], bf16, tag="bv")
            nc.sync.dma_start(out=bv[:], in_=bv_d[:])
            ident = pp.tile([P, P], bf16, tag="ident")
            nc.sync.dma_start(out=ident[:], in_=ident_d[:])
            xT = pp.tile([P, CT * NL], bf16, tag="xT")
            nc.sync.dma_start(out=xT[:], in_=xT_d[:])
            wk = wp.tile([P, CT * C], bf16, tag="W", name="wk")
            nc.sync.dma_start(out=wk[:, 0:4 * C], in_=Wk_d[:, 0:4 * C])
            nc.sync.dma_start(out=wk[:, 4 * C:], in_=Wk_d[:, 4 * C:])
            wu = wp.tile([P, CT * C], bf16, tag="W", name="wu")
            nc.sync.dma_start(out=wu[:], in_=Wu_d[:])
            # wq reuses a W slot (WAR on k-proj completion)
            wq = wp.tile([P, CT * C], bf16, tag="W", name="wq")
            nc.sync.dma_start(out=wq[:], in_=Wq_d[:])

            qT = pp.tile([P, CT * NL], bf16, tag="qT")
            expS = pp.tile([P, (N // P) * NL], bf16, tag="expS")
            # k staging, chunk-major: kst[:, h*CT*w + co*w + m]
            kst = pp.tile([P, NCH * CT * 2 * P], bf16, tag="kst")
            # v staging, row-block-major: vst[:, nt*C + c] = v[nt*P+p, c]
            vst = pp.tile([P, (NL // P) * C], bf16, tag="vst")
            oT = pp.tile([P, CT * NL], bf16, tag="oT")

            # ---- AG bounce buffers ----
            agv_in = dp.tile([NL, C], bf16, tag="agv_in")
            agk_in = []
            agk_out = []
            for h, (st0, nt) in enumerate(KCHUNKS):
                w = nt * P
                ki = dp.tile([P, CT * w], bf16, tag=f"agk_in{h}",
                             name=f"agk_in{h}")
                agk_in.append(ki)
                ko = dp.tile([R * P, CT * w], bf16, addr_space="Shared",
                             tag=f"agk_out{h}", name=f"agk_out{h}")
                agk_out.append(ko)
            agv_out = []
            for h, (st0, nt) in enumerate(VCHUNKS):
                vo = dp.tile([R * nt * P, C], bf16, addr_space="Shared",
                             tag=f"agv_out{h}", name=f"agv_out{h}")
                agv_out.append(vo)

            # ---- phase A: projections (ci-outer, 8 PSUM banks) ----
            with tc.tile_pool(name="pa", bufs=CT, space="PSUM") as pa:
                # k.T [c_out, n]
                kps = []
                for co in range(CT):
                    kco = pa.tile([P, NL], f32, tag="pa", name=f"kps{co}")
                    kps.append(kco)
                for ci in range(CT):
                    for co in range(CT):
                        nc.tensor.matmul(
                            kps[co][:],
                            lhsT=wk[:, ci * C + co * P:ci * C + (co + 1) * P],
                            rhs=xT[:, ci * NL:(ci + 1) * NL],
                            start=(ci == 0), stop=(ci == CT - 1),
                            skip_group_check=True,
                        )
                # bias + bf16, written chunk-major so each AG chunk is one
                # contiguous staging DMA
                for co in range(CT):
                    for h, (st0, nt) in enumerate(KCHUNKS):
                        w = nt * P
                        nc.scalar.activation(
                            kst[:, h * CT * w + co * w:
                                h * CT * w + (co + 1) * w],
                            kps[co][:, st0 * P:st0 * P + w],
                            Ident, bias=cst[:, 8 + co:9 + co])
                for h, (st0, nt) in enumerate(KCHUNKS):
                    w = nt * P
                    nc.sync.dma_start(
                        out=agk_in[h][:],
                        in_=kst[:, h * CT * w:(h + 1) * CT * w])

                for h in range(NCH):
                    nc.gpsimd.collective_compute(
                        "AllGather", mybir.AluOpType.bypass,
                        replica_groups=rg,
                        ins=[agk_in[h][:]], outs=[agk_out[h][:]],
                    )

                # v [n, c_out]: bias via ones-row matmul; vps[i] covers
                # v rows [nt*P,(nt+1)*P) cols [ch*NL,(ch+1)*NL), i=nt*2+ch
                vps = []
                for i in range(CT):
                    vpi = pa.tile([P, NL], f32, tag="pa", name=f"vps{i}")
                    vps.append(vpi)
                for i in range(CT):
                    ch = i % 2
                    nc.tensor.matmul(
                        vps[i][:], lhsT=bones[0:1, 0:P],
                        rhs=bv[0:1, ch * NL:(ch + 1) * NL],
                        start=True, stop=False, skip_group_check=True,
                    )
                for ci in range(CT):
                    for i in range(CT):
                        nt, ch = i // 2, i % 2
                        nc.tensor.matmul(
                            vps[i][:],
                            lhsT=xT[:, ci * NL + nt * P:ci * NL + (nt + 1) * P],
                            rhs=wu[:, ci * C + ch * NL:ci * C + (ch + 1) * NL],
                            start=False, stop=(ci == CT - 1),
                            skip_group_check=True,
                        )
                for i in range(CT):
                    nt, ch = i // 2, i % 2
                    nc.vector.tensor_copy(
                        vst[:, nt * C + ch * NL:nt * C + (ch + 1) * NL],
                        vps[i][:])
                # one staging DMA per chunk via rearranged dram AP
                agv_r = agv_in[:].rearrange("(b p) c -> p b c", p=P)
                for h, (st0, nt) in enumerate(VCHUNKS):
                    nc.sync.dma_start(
                        out=agv_r[:, st0:st0 + nt, :],
                        in_=vst[:, st0 * C:(st0 + nt) * C])

                for h, (st0, nt) in enumerate(VCHUNKS):
                    nc.gpsimd.collective_compute(
                        "AllGather", mybir.AluOpType.bypass,
                        replica_groups=rg,
                        ins=[agv_in[st0 * P:(st0 + nt) * P, :]],
                        outs=[agv_out[h][:]],
                    )

                # q.T [c_out, n]
                qps = []
                for co in range(CT):
                    qco = pa.tile([P, NL], f32, tag="pa", name=f"qps{co}")
                    qps.append(qco)
                for ci in range(CT):
                    for co in range(CT):
                        nc.tensor.matmul(
                            qps[co][:],
                            lhsT=wq[:, ci * C + co * P:ci * C + (co + 1) * P],
                            rhs=xT[:, ci * NL:(ci + 1) * NL],
                            start=(ci == 0), stop=(ci == CT - 1),
                            skip_group_check=True,
                        )
                for co in range(CT):
                    nc.scalar.activation(qT[:, co * NL:(co + 1) * NL],
                                         qps[co][:], Ident,
                                         bias=cst[:, co:co + 1])

                # HAM keep-warm: junk matmuls reading qT (so they schedule
                # after q-proj) fill the PE-idle window while AG_k0 is in
                # flight; without them the clock gate drops to 4/8 and the
                # first ~3.4us of phase S run at half rate.
                jk0 = pa.tile([P, NL], f32, tag="pa", name="junk0")
                jk1 = pa.tile([P, NL], f32, tag="pa", name="junk1")
                for i in range(100):
                    nc.tensor.matmul(
                        (jk0 if i % 2 == 0 else jk1)[:],
                        lhsT=bones[:, 0:P], rhs=qT[:, 0:NL],
                        start=True, stop=True, skip_group_check=True,
                    )

            # ---- phase S: S.T tiles + exp, chunk by chunk; 4-way PSUM
            # interleave (5 banks) + 2 row-sum banks + 1 bcast bank ----
            bcast_sb = pp.tile([P, NL], bf16, tag="bcast")
            nslice = R * sum(nt for _, nt in KCHUNKS)
            with tc.tile_pool(name="ps", bufs=6, space="PSUM") as psp:
                # row sums ride on the idle DVE: racc += expS_t per tile;
                # the cross-partition reduce + 1/x broadcast then run on
                # GpSimd/DVE with no PSUM footprint, so phase AV's 8
                # accumulator banks have no WAR on this chain
                racc = pp.tile([P, NL], f32, tag="racc")

                def emit_rowsums(ts):
                    for t in ts:
                        if t == 0:
                            nc.vector.tensor_copy(
                                racc[:], expS[:, 0:NL])
                        else:
                            nc.vector.tensor_add(
                                racc[:], racc[:],
                                expS[:, t * NL:(t + 1) * NL])

                pending = []
                kt_cache = {}

                def get_kt(h, j, w):
                    key = (h, j)
                    if key not in kt_cache:
                        kt = ktp.tile([P, CT * w], bf16, tag="kt",
                                      name=f"kt{h}_{j}")
                        # first tile of each chunk loads in quarters so
                        # phase S starts ~1.5us after the AG completes
                        # (keep ALL kt loads on the sync ring: a second
                        # HWDGE ring head-of-line-blocks exp ACTs)
                        nparts = 4 if j == 0 else 2
                        step = CT * w // nparts
                        for u in range(nparts):
                            nc.sync.dma_start(
                                out=kt[:, u * step:(u + 1) * step],
                                in_=agk_out[h][j * P:(j + 1) * P,
                                               u * step:(u + 1) * step])
                        kt_cache[key] = kt
                    return kt_cache[key]

                tiles = []
                for h, (st0, nt) in enumerate(KCHUNKS):
                    for j in range(R):
                        for mh in range(nt):
                            tiles.append((h, j, mh, nt))
                for g0 in range(0, len(tiles), 4):
                    group = tiles[g0:g0 + 4]
                    pss = []
                    for gi, (h, j, mh, nt) in enumerate(group):
                        ps = psp.tile([P, NL], f32, tag="ps",
                                      name=f"ps{g0 + gi}")
                        pss.append(ps)
                    for ci in range(CT):
                        for gi, (h, j, mh, nt) in enumerate(group):
                            w = nt * P
                            kt = get_kt(h, j, w)
                            nc.tensor.matmul(
                                pss[gi][:],
                                lhsT=kt[:, ci * w + mh * P:
                                        ci * w + (mh + 1) * P],
                                rhs=qT[:, ci * NL:(ci + 1) * NL],
                                start=(ci == 0), stop=(ci == CT - 1),
                                skip_group_check=True,
                            )
                    emit_rowsums(pending)
                    pending = []
                    for gi in range(len(group)):
                        nc.scalar.activation(
                            expS[:, (g0 + gi) * NL:(g0 + gi + 1) * NL],
                            pss[gi][:], Exp, bias=cst[:, 24:25])
                        pending.append(g0 + gi)
                emit_rowsums(pending)

                # partition-reduce racc on GpSimd, reciprocal on DVE,
                # broadcast back across partitions on GpSimd
                rsum = pp.tile([P, NL], f32, tag="rsum")
                nc.gpsimd.partition_all_reduce(
                    rsum[:], racc[:], channels=P,
                    reduce_op=bass_isa.ReduceOp.add)
                recip = pp.tile([1, NL], f32, tag="recip")
                nc.vector.reciprocal(recip[:], rsum[0:1, :])
                recip_bf = pp.tile([1, NL], bf16, tag="recipb")
                nc.vector.tensor_copy(recip_bf[:], recip[:])
                nc.gpsimd.partition_broadcast(bcast_sb[:], recip_bf[:])
                # residual pre-scale on the idle DVE, mid-AV:
                # xpbrs = (x.T + bo) * rowsum, so adding it to the hps
                # accumulators (identity matmuls below) makes the final
                # out = hps_total * (1/rowsum) -- the epilogue collapses
                # to one DVE mul per co
                xpbrs = pp.tile([P, CT * NL], bf16, tag="xpbrs")
                for co in range(CT):
                    nc.vector.scalar_tensor_tensor(
                        xpbrs[:, co * NL:(co + 1) * NL],
                        xT[:, co * NL:(co + 1) * NL],
                        cst[:, 16 + co:17 + co], rsum[:],
                        mybir.AluOpType.add, mybir.AluOpType.mult)

            # ---- phase AV: h.T accumulation, chunk by chunk ----
            with tc.tile_pool(name="ph", bufs=CT, space="PSUM") as ph:
                hps = []
                for co in range(CT):
                    hco = ph.tile([P, NL], f32, tag="h", name=f"h{co}")
                    hps.append(hco)
                # vt loads as 1MB quads (4 nj-tiles each) on the GpSimd
                # SWDGE queue so they can't head-block the sync queue.
                # v-chunk tile (vj, vst0+vm) pairs with expS slice
                # _t_of(vj, vst0+vm) (k-chunk-major production order).
                idx = 0
                for h, (st0, nt) in enumerate(VCHUNKS):
                    agv_3 = agv_out[h][:].rearrange("(b p) c -> p b c", p=P)
                    nquad = (R * nt) // 4
                    for qq in range(nquad):
                        vq = vtp.tile([P, 4 * C], bf16, tag="vt",
                                      name=f"vq{h}_{qq}")
                        nc.gpsimd.dma_start(
                            out=vq[:, 0:2 * C],
                            in_=agv_3[:, qq * 4:qq * 4 + 2, :])
                        nc.gpsimd.dma_start(
                            out=vq[:, 2 * C:],
                            in_=agv_3[:, qq * 4 + 2:qq * 4 + 4, :])
                        for m in range(4):
                            rowt = qq * 4 + m      # row-tile within chunk
                            vj, vm = rowt // nt, rowt % nt
                            t = _t_of(vj, st0 + vm)
                            for co in range(CT):
                                nc.tensor.matmul(
                                    hps[co][:],
                                    lhsT=vq[:, m * C + co * P:
                                            m * C + (co + 1) * P],
                                    rhs=expS[:, t * NL:(t + 1) * NL],
                                    start=(idx == 0),
                                    stop=False,
                                    skip_group_check=True,
                                )
                                if idx == nslice - 1:
                                    # fold the pre-scaled residual in with
                                    # an identity matmul, interleaved so
                                    # bank co's drain (and its epilogue
                                    # mul) starts as early as possible
                                    nc.tensor.matmul(
                                        hps[co][:], lhsT=ident[:],
                                        rhs=xpbrs[:, co * NL:(co + 1) * NL],
                                        start=False, stop=True,
                                        skip_group_check=True,
                                    )
                            idx += 1
                # epilogue: out.T = hps_total * (1/rowsum), per co
                outT_r = outT_d.rearrange("(b p) n -> p b n", p=P)
                for co in range(CT):
                    nc.vector.tensor_mul(oT[:, co * NL:(co + 1) * NL],
                                         hps[co][:], bcast_sb[:])
                    nc.sync.dma_start(
                        out=outT_r[:, co:co + 1, :],
                        in_=oT[:, co * NL:(co + 1) * NL])

    nc.compile()
    return nc


def kernel(x, Wq, bq, Wk, bk, Wv, bv, Wo, bo):
    x = np.ascontiguousarray(np.asarray(x, dtype=np.float32))

    if "nc" not in _CACHE:
        _CACHE["nc"] = _build()
    nc = _CACHE["nc"]

    def wtile(a):  # [C_out, C_in] -> [P, CT*C] lhsT-tiled (bf16)
        wt = np.asarray(a, np.float32).T  # [C_in, C_out]
        return np.ascontiguousarray(
            wt.reshape(CT, P, C).transpose(1, 0, 2).reshape(P, CT * C)
        ).astype(npbf)

    cstv = np.zeros((P, 160), np.float32)
    cstv[:, 0:8] = np.asarray(bq, np.float32).reshape(CT, P).T
    cstv[:, 8:16] = np.asarray(bk, np.float32).reshape(CT, P).T
    cstv[:, 16:24] = np.asarray(bo, np.float32).reshape(CT, P).T
    cstv[:, 24] = SHIFT
    cstv[:, 32:160] = 1.0

    # output-projection folding: u = x@Wu.T + bu with Wu = Wo@Wv,
    # bu = Wo@bv, so attn@u == (attn@v)@Wo.T (associativity)
    Wu = np.asarray(Wo, np.float32) @ np.asarray(Wv, np.float32)
    bu = np.asarray(Wo, np.float32) @ np.asarray(bv, np.float32)
    shared = {
        "Wq2": wtile(Wq), "Wk2": wtile(Wk), "Wu2": wtile(Wu),
        "cst": cstv,
        "bones": np.ones((P, NL), npbf),
        "bvrow": bu.reshape(1, C).astype(npbf),
        "ident": np.eye(P, dtype=np.float32).astype(npbf),
    }
    in_maps = []
    for i in range(R):
        m = dict(shared)
        xs = x[i * NL:(i + 1) * NL, :]  # [NL, C]
        m["xT"] = np.ascontiguousarray(
            xs.T.reshape(CT, P, NL).transpose(1, 0, 2).reshape(P, CT * NL)
        ).astype(npbf)
        in_maps.append(m)

    res = run_bass_kernel_spmd(nc, in_maps, core_ids=list(range(R)),
                               trace=TRACE)
    _CACHE["last_result"] = res

    out = np.empty((N, C), dtype=np.float32)
    for i in range(R):
        out[i * NL:(i + 1) * NL, :] = \
            res.results[i]["outT"].T.astype(np.float32)
    return out


# revision 57
# speedup vs baseline: 1.0638x; 1.0638x over previous
"""Distributed attention block on 8 TRN2 NeuronCores.

Reference math (torch Linear convention, no 1/sqrt(d) scale):
    q = x @ Wq.T + bq ; k = x @ Wk.T + bk ; v = x @ Wv.T + bv
    attn = softmax(q @ k.T, axis=-1)
    out = x + (attn @ v) @ Wo.T + bo

Output-projection folding: (attn @ v) @ Wo.T == attn @ (v @ Wo.T),
and v @ Wo.T = x @ (Wo Wv).T + Wo bv, so the kernel computes
u = x @ Wu.T + bu with host-premultiplied Wu = Wo @ Wv, bu = Wo @ bv,
gathers u instead of v, and finishes with out = x + attn @ u + bo.
This moves the entire output projection off the post-gather critical
tail and into phase A, where the PE would otherwise idle waiting for
the collective queue.

Sharding: rows of x (N=4096) split across 8 cores (512 rows each).
Each core computes its q tile; k and u are all-gathered in 2 chunks
each (k chunks first) so S starts when the first k chunk lands, and
attn@u consumes slices in the same chunk-major order.

Everything on-chip is computed in transposed layout ([C, n] feature
major) so biases are per-partition and QK^T is produced directly as
S.T (nj on partitions), which feeds attn@v without transposes. The
softmax denominators accumulate on the idle DVE (racc += expS tile),
partition-reduce + 1/x-broadcast on GpSimd/DVE, keeping the PE stream
pure matmul and all 8 PSUM banks free for the attn@v accumulators.

Key perf structure (HWDGE dispatch costs ~0.7us per dma_start on the
issuing engine's queue, so DMA count is minimized; the NRT collective
start barrier is a fixed ~21+35-65us from kernel start and each AG op
has a ~18.5us floor, so the chain is 4 ops of >=512KB):
  - weights / xT / consts each load as one or two big DMAs; W tiles
    live in a 2-slot pool (Wk dies after k-proj).
  - k staging is chunk-major in SBUF so each AG chunk stages with ONE
    contiguous DMA; u staging uses one rearranged-AP DMA per chunk.
  - kt gather readback on the sync queue per [128, .] half-tile; u
    readback as 1MB quads on the GpSimd SWDGE queue (so neither can
    head-of-line-block the other's dispatch queue).
  - ~100 junk matmuls on qT keep the PE clock gate (HAM) at 8/8
    through the idle window while AG_k0 is in flight.
  - residual uses the bf16 xT already in SBUF (no fp32 x copy).

Compute dtype bf16 (PSUM accumulation fp32). A global shift of -40 is
applied inside exp(): softmax is shift-invariant, the global logit max
~79 would otherwise ride close to fp32 overflow, and every row max is
>= 39.8 so denominators stay O(1).
"""

import numpy as np
import ml_dtypes

import concourse.bass as bass
import concourse.tile as tile
from concourse import bacc, bass_isa, mybir
from concourse.bass_utils import run_bass_kernel_spmd

N = 4096
C = 1024
R = 8            # cores
NL = N // R      # 512 rows per core
P = 128
CT = C // P      # 8 c tiles
KCHUNKS = [(0, 2), (2, 2)]   # (start nj-tile, n nj-tiles) per k AG chunk
NCH = len(KCHUNKS)
# v chunks: measured per-op floor is ~18.5us so fewer/bigger ops win
VCHUNKS = [(0, 2), (2, 2)]


def _t_of(j, m):
    """expS slice index for global nj-tile (rank j, tile m within rank),
    matching the k-chunk-major S production order."""
    for h, (st0, nt) in enumerate(KCHUNKS):
        if st0 <= m < st0 + nt:
            base = sum(R * n for _, n in KCHUNKS[:h])
            return base + j * nt + (m - st0)
    raise ValueError(m)
SHIFT = -40.0    # global logit shift inside exp

f32 = mybir.dt.float32
bf16 = mybir.dt.bfloat16
npbf = ml_dtypes.bfloat16

TRACE = False
_CACHE = {}


def _build():
    nc = bacc.Bacc("TRN2", target_bir_lowering=False, debug=False,
                   num_devices=R)

    # host-prepped layouts (see kernel() below)
    xT_d = nc.dram_tensor("xT", [P, CT * NL], bf16, kind="ExternalInput").ap()
    Wk_d = nc.dram_tensor("Wk2", [P, CT * C], bf16, kind="ExternalInput").ap()
    Wu_d = nc.dram_tensor("Wu2", [P, CT * C], bf16, kind="ExternalInput").ap()
    Wq_d = nc.dram_tensor("Wq2", [P, CT * C], bf16, kind="ExternalInput").ap()
    # [:, 0:8]=bqc  [:, 8:16]=bkc  [:, 16:24]=boc  [:, 24]=shift
    # [:, 32:160]=1.0f (ones row for the fp32 broadcast matmul)
    cst_d = nc.dram_tensor("cst", [P, 160], f32, kind="ExternalInput").ap()
    bones_d = nc.dram_tensor("bones", [P, NL], bf16, kind="ExternalInput").ap()
    bv_d = nc.dram_tensor("bvrow", [1, C], bf16, kind="ExternalInput").ap()
    ident_d = nc.dram_tensor("ident", [P, P], bf16, kind="ExternalInput").ap()
    outT_d = nc.dram_tensor("outT", [C, NL], bf16,
                            kind="ExternalOutput").ap()

    Exp = mybir.ActivationFunctionType.Exp
    Ident = mybir.ActivationFunctionType.Identity
    rg = [list(range(R))]

    with tile.TileContext(nc) as tc:
        with (
            tc.tile_pool(name="persist", bufs=1) as pp,
            tc.tile_pool(name="wpool", bufs=2) as wp,
            tc.tile_pool(name="ktp", bufs=10) as ktp,
            tc.tile_pool(name="vtp", bufs=4) as vtp,
            tc.tile_pool(name="dram", bufs=1, space="DRAM") as dp,
        ):
            # ---- front-loaded DMAs (sync queue; ~0.7us dispatch each) ----
            cst = pp.tile([P, 160], f32, tag="cst")
            nc.sync.dma_start(out=cst[:], in_=cst_d[:])
            bones = pp.tile([P, NL], bf16, tag="bones")
            nc.sync.dma_start(out=bones[:], in_=bones_d[:])
            bv = pp.tile([1, C], bf16, tag="bv")
            nc.sync.dma_start(out=bv[:], in_=bv_d[:])
            ident = pp.tile([P, P], bf16, tag="ident")
            nc.sync.dma_start(out=ident[:], in_=ident_d[:])
            xT = pp.tile([P, CT * NL], bf16, tag="xT")
            nc.sync.dma_start(out=xT[:], in_=xT_d[:])
            wk = wp.tile([P, CT * C], bf16, tag="W", name="wk")
            nc.sync.dma_start(out=wk[:, 0:4 * C], in_=Wk_d[:, 0:4 * C])
            nc.sync.dma_start(out=wk[:, 4 * C:], in_=Wk_d[:, 4 * C:])
            wu = wp.tile([P, CT * C], bf16, tag="W", name="wu")
            nc.sync.dma_start(out=wu[:], in_=Wu_d[:])
            # wq reuses a W slot (WAR on k-proj completion)
            wq = wp.tile([P, CT * C], bf16, tag="W", name="wq")
            nc.sync.dma_start(out=wq[:], in_=Wq_d[:])

            qT = pp.tile([P, CT * NL], bf16, tag="qT")
            expS = pp.tile([P, (N // P) * NL], bf16, tag="expS")
            # k staging, chunk-major: kst[:, h*CT*w + co*w + m]
            kst = pp.tile([P, NCH * CT * 2 * P], bf16, tag="kst")
            # v staging, row-block-major: vst[:, nt*C + c] = v[nt*P+p, c]
            vst = pp.tile([P, (NL // P) * C], bf16, tag="vst")
            oT = pp.tile([P, CT * NL], bf16, tag="oT")

            # ---- AG bounce buffers ----
            agv_in = dp.tile([NL, C], bf16, tag="agv_in")
            agk_in = []
            agk_out = []
            for h, (st0, nt) in enumerate(KCHUNKS):
                w = nt * P
                ki = dp.tile([P, CT * w], bf16, tag=f"agk_in{h}",
                             name=f"agk_in{h}")
                agk_in.append(ki)
                ko = dp.tile([R * P, CT * w], bf16, addr_space="Shared",
                             tag=f"agk_out{h}", name=f"agk_out{h}")
                agk_out.append(ko)
            agv_out = []
            for h, (st0, nt) in enumerate(VCHUNKS):
                vo = dp.tile([R * nt * P, C], bf16, addr_space="Shared",
                             tag=f"agv_out{h}", name=f"agv_out{h}")
                agv_out.append(vo)

            # ---- phase A: projections (ci-outer, 8 PSUM banks) ----
            with tc.tile_pool(name="pa", bufs=CT, space="PSUM") as pa:
                # k.T [c_out, n]
                kps = []
                for co in range(CT):
                    kco = pa.tile([P, NL], f32, tag="pa", name=f"kps{co}")
                    kps.append(kco)
                for ci in range(CT):
                    for co in range(CT):
                        nc.tensor.matmul(
                            kps[co][:],
                            lhsT=wk[:, ci * C + co * P:ci * C + (co + 1) * P],
                            rhs=xT[:, ci * NL:(ci + 1) * NL],
                            start=(ci == 0), stop=(ci == CT - 1),
                            skip_group_check=True,
                        )
                # bias + bf16, written chunk-major so each AG chunk is one
                # contiguous staging DMA
                for co in range(CT):
                    for h, (st0, nt) in enumerate(KCHUNKS):
                        w = nt * P
                        nc.scalar.activation(
                            kst[:, h * CT * w + co * w:
                                h * CT * w + (co + 1) * w],
                            kps[co][:, st0 * P:st0 * P + w],
                            Ident, bias=cst[:, 8 + co:9 + co])
                for h, (st0, nt) in enumerate(KCHUNKS):
                    w = nt * P
                    nc.sync.dma_start(
                        out=agk_in[h][:],
                        in_=kst[:, h * CT * w:(h + 1) * CT * w])

                for h in range(NCH):
                    nc.gpsimd.collective_compute(
                        "AllGather", mybir.AluOpType.bypass,
                        replica_groups=rg,
                        ins=[agk_in[h][:]], outs=[agk_out[h][:]],
                    )

                # v [n, c_out]: bias via ones-row matmul; vps[i] covers
                # v rows [nt*P,(nt+1)*P) cols [ch*NL,(ch+1)*NL), i=nt*2+ch
                vps = []
                for i in range(CT):
                    vpi = pa.tile([P, NL], f32, tag="pa", name=f"vps{i}")
                    vps.append(vpi)
                for i in range(CT):
                    ch = i % 2
                    nc.tensor.matmul(
                        vps[i][:], lhsT=bones[0:1, 0:P],
                        rhs=bv[0:1, ch * NL:(ch + 1) * NL],
                        start=True, stop=False, skip_group_check=True,
                    )
                for ci in range(CT):
                    for i in range(CT):
                        nt, ch = i // 2, i % 2
                        nc.tensor.matmul(
                            vps[i][:],
                            lhsT=xT[:, ci * NL + nt * P:ci * NL + (nt + 1) * P],
                            rhs=wu[:, ci * C + ch * NL:ci * C + (ch + 1) * NL],
                            start=False, stop=(ci == CT - 1),
                            skip_group_check=True,
                        )
                for i in range(CT):
                    nt, ch = i // 2, i % 2
                    nc.vector.tensor_copy(
                        vst[:, nt * C + ch * NL:nt * C + (ch + 1) * NL],
                        vps[i][:])
                # one staging DMA per chunk via rearranged dram AP
                agv_r = agv_in[:].rearrange("(b p) c -> p b c", p=P)
                for h, (st0, nt) in enumerate(VCHUNKS):
                    nc.sync.dma_start(
                        out=agv_r[:, st0:st0 + nt, :],
                        in_=vst[:, st0 * C:(st0 + nt) * C])

                for h, (st0, nt) in enumerate(VCHUNKS):
                    nc.gpsimd.collective_compute(
                        "AllGather", mybir.AluOpType.bypass,
                        replica_groups=rg,
                        ins=[agv_in[st0 * P:(st0 + nt) * P, :]],
                        outs=[agv_out[h][:]],
                    )

                # q.T [c_out, n]
                qps = []
                for co in range(CT):
                    qco = pa.tile([P, NL], f32, tag="pa", name=f"qps{co}")
                    qps.append(qco)
                for ci in range(CT):
                    for co in range(CT):
                        nc.tensor.matmul(
                            qps[co][:],
                            lhsT=wq[:, ci * C + co * P:ci * C + (co + 1) * P],
                            rhs=xT[:, ci * NL:(ci + 1) * NL],
                            start=(ci == 0), stop=(ci == CT - 1),
                            skip_group_check=True,
                        )
                for co in range(CT):
                    nc.scalar.activation(qT[:, co * NL:(co + 1) * NL],
                                         qps[co][:], Ident,
                                         bias=cst[:, co:co + 1])

                # HAM keep-warm: junk matmuls reading qT (so they schedule
                # after q-proj) fill the PE-idle window while AG_k0 is in
                # flight; without them the clock gate drops to 4/8 and the
                # first ~3.4us of phase S run at half rate.
                jk0 = pa.tile([P, NL], f32, tag="pa", name="junk0")
                jk1 = pa.tile([P, NL], f32, tag="pa", name="junk1")
                for i in range(100):
                    nc.tensor.matmul(
                        (jk0 if i % 2 == 0 else jk1)[:],
                        lhsT=bones[:, 0:P], rhs=qT[:, 0:NL],
                        start=True, stop=True, skip_group_check=True,
                    )

            # ---- phase S: S.T tiles + exp, chunk by chunk; 4-way PSUM
            # interleave (5 banks) + 2 row-sum banks + 1 bcast bank ----
            bcast_sb = pp.tile([P, NL], bf16, tag="bcast")
            nslice = R * sum(nt for _, nt in KCHUNKS)
            with tc.tile_pool(name="ps", bufs=6, space="PSUM") as psp:
                # row sums ride on the idle DVE: racc += expS_t per tile;
                # the cross-partition reduce + 1/x broadcast then run on
                # GpSimd/DVE with no PSUM footprint, so phase AV's 8
                # accumulator banks have no WAR on this chain
                racc = pp.tile([P, NL], f32, tag="racc")

                def emit_rowsums(ts):
                    for t in ts:
                        if t == 0:
                            nc.vector.tensor_copy(
                                racc[:], expS[:, 0:NL])
                        else:
                            nc.vector.tensor_add(
                                racc[:], racc[:],
                                expS[:, t * NL:(t + 1) * NL])

                pending = []
                kt_cache = {}

                def get_kt(h, j, w):
                    key = (h, j)
                    if key not in kt_cache:
                        kt = ktp.tile([P, CT * w], bf16, tag="kt",
                                      name=f"kt{h}_{j}")
                        # first two tiles of each chunk load in quarters
                        # so phase S starts right after the AG completes
                        # (keep ALL kt loads on the sync ring: a second
                        # HWDGE ring head-of-line-blocks exp ACTs)
                        nparts = 4 if j < 2 else 2
                        step = CT * w // nparts
                        for u in range(nparts):
                            nc.sync.dma_start(
                                out=kt[:, u * step:(u + 1) * step],
                                in_=agk_out[h][j * P:(j + 1) * P,
                                               u * step:(u + 1) * step])
                        kt_cache[key] = kt
                    return kt_cache[key]

                tiles = []
                for h, (st0, nt) in enumerate(KCHUNKS):
                    for j in range(R):
                        for mh in range(nt):
                            tiles.append((h, j, mh, nt))
                # the first two groups hold 2 tiles each (one kt tile per
                # group), so the first matmuls gate on a single quarter-
                # DMA of kt(h,0) instead of two full tiles
                sizes = [2, 2] + [4] * ((len(tiles) - 4) // 4)
                g0 = 0
                for size in sizes:
                    group = tiles[g0:g0 + size]
                    pss = []
                    for gi, (h, j, mh, nt) in enumerate(group):
                        ps = psp.tile([P, NL], f32, tag="ps",
                                      name=f"ps{g0 + gi}")
                        pss.append(ps)
                    for ci in range(CT):
                        for gi, (h, j, mh, nt) in enumerate(group):
                            w = nt * P
                            kt = get_kt(h, j, w)
                            nc.tensor.matmul(
                                pss[gi][:],
                                lhsT=kt[:, ci * w + mh * P:
                                        ci * w + (mh + 1) * P],
                                rhs=qT[:, ci * NL:(ci + 1) * NL],
                                start=(ci == 0), stop=(ci == CT - 1),
                                skip_group_check=True,
                            )
                    emit_rowsums(pending)
                    pending = []
                    for gi in range(len(group)):
                        nc.scalar.activation(
                            expS[:, (g0 + gi) * NL:(g0 + gi + 1) * NL],
                            pss[gi][:], Exp, bias=cst[:, 24:25])
                        pending.append(g0 + gi)
                    g0 += size
                emit_rowsums(pending)

                # partition-reduce racc on GpSimd, reciprocal on DVE,
                # broadcast back across partitions on GpSimd
                rsum = pp.tile([P, NL], f32, tag="rsum")
                nc.gpsimd.partition_all_reduce(
                    rsum[:], racc[:], channels=P,
                    reduce_op=bass_isa.ReduceOp.add)
                recip = pp.tile([1, NL], f32, tag="recip")
                nc.vector.reciprocal(recip[:], rsum[0:1, :])
                recip_bf = pp.tile([1, NL], bf16, tag="recipb")
                nc.vector.tensor_copy(recip_bf[:], recip[:])
                nc.gpsimd.partition_broadcast(bcast_sb[:], recip_bf[:])
                # residual pre-scale on the idle DVE, mid-AV:
                # xpbrs = (x.T + bo) * rowsum, so adding it to the hps
                # accumulators (identity matmuls below) makes the final
                # out = hps_total * (1/rowsum) -- the epilogue collapses
                # to one DVE mul per co
                xpbrs = pp.tile([P, CT * NL], bf16, tag="xpbrs")
                for co in range(CT):
                    nc.vector.scalar_tensor_tensor(
                        xpbrs[:, co * NL:(co + 1) * NL],
                        xT[:, co * NL:(co + 1) * NL],
                        cst[:, 16 + co:17 + co], rsum[:],
                        mybir.AluOpType.add, mybir.AluOpType.mult)

            # ---- phase AV: h.T accumulation, chunk by chunk ----
            with tc.tile_pool(name="ph", bufs=CT, space="PSUM") as ph:
                hps = []
                for co in range(CT):
                    hco = ph.tile([P, NL], f32, tag="h", name=f"h{co}")
                    hps.append(hco)
                # vt loads as 1MB quads (4 nj-tiles each) on the GpSimd
                # SWDGE queue so they can't head-block the sync queue.
                # v-chunk tile (vj, vst0+vm) pairs with expS slice
                # _t_of(vj, vst0+vm) (k-chunk-major production order).
                idx = 0
                for h, (st0, nt) in enumerate(VCHUNKS):
                    agv_3 = agv_out[h][:].rearrange("(b p) c -> p b c", p=P)
                    nquad = (R * nt) // 4
                    for qq in range(nquad):
                        vq = vtp.tile([P, 4 * C], bf16, tag="vt",
                                      name=f"vq{h}_{qq}")
                        nc.gpsimd.dma_start(
                            out=vq[:, 0:2 * C],
                            in_=agv_3[:, qq * 4:qq * 4 + 2, :])
                        nc.gpsimd.dma_start(
                            out=vq[:, 2 * C:],
                            in_=agv_3[:, qq * 4 + 2:qq * 4 + 4, :])
                        for m in range(4):
                            rowt = qq * 4 + m      # row-tile within chunk
                            vj, vm = rowt // nt, rowt % nt
                            t = _t_of(vj, st0 + vm)
                            for co in range(CT):
                                nc.tensor.matmul(
                                    hps[co][:],
                                    lhsT=vq[:, m * C + co * P:
                                            m * C + (co + 1) * P],
                                    rhs=expS[:, t * NL:(t + 1) * NL],
                                    start=(idx == 0),
                                    stop=False,
                                    skip_group_check=True,
                                )
                                if idx == nslice - 1:
                                    # fold the pre-scaled residual in with
                                    # an identity matmul, interleaved so
                                    # bank co's drain (and its epilogue
                                    # mul) starts as early as possible
                                    nc.tensor.matmul(
                                        hps[co][:], lhsT=ident[:],
                                        rhs=xpbrs[:, co * NL:(co + 1) * NL],
                                        start=False, stop=True,
                                        skip_group_check=True,
                                    )
                            idx += 1
                # epilogue: out.T = hps_total * (1/rowsum), per co
                outT_r = outT_d.rearrange("(b p) n -> p b n", p=P)
                for co in range(CT):
                    nc.vector.tensor_mul(oT[:, co * NL:(co + 1) * NL],
                                         hps[co][:], bcast_sb[:])
                    nc.sync.dma_start(
                        out=outT_r[:, co:co + 1, :],
                        in_=oT[:, co * NL:(co + 1) * NL])

    nc.compile()
    return nc


def kernel(x, Wq, bq, Wk, bk, Wv, bv, Wo, bo):
    x = np.ascontiguousarray(np.asarray(x, dtype=np.float32))

    if "nc" not in _CACHE:
        _CACHE["nc"] = _build()
    nc = _CACHE["nc"]

    def wtile(a):  # [C_out, C_in] -> [P, CT*C] lhsT-tiled (bf16)
        wt = np.asarray(a, np.float32).T  # [C_in, C_out]
        return np.ascontiguousarray(
            wt.reshape(CT, P, C).transpose(1, 0, 2).reshape(P, CT * C)
        ).astype(npbf)

    cstv = np.zeros((P, 160), np.float32)
    cstv[:, 0:8] = np.asarray(bq, np.float32).reshape(CT, P).T
    cstv[:, 8:16] = np.asarray(bk, np.float32).reshape(CT, P).T
    cstv[:, 16:24] = np.asarray(bo, np.float32).reshape(CT, P).T
    cstv[:, 24] = SHIFT
    cstv[:, 32:160] = 1.0

    # output-projection folding: u = x@Wu.T + bu with Wu = Wo@Wv,
    # bu = Wo@bv, so attn@u == (attn@v)@Wo.T (associativity)
    Wu = np.asarray(Wo, np.float32) @ np.asarray(Wv, np.float32)
    bu = np.asarray(Wo, np.float32) @ np.asarray(bv, np.float32)
    shared = {
        "Wq2": wtile(Wq), "Wk2": wtile(Wk), "Wu2": wtile(Wu),
        "cst": cstv,
        "bones": np.ones((P, NL), npbf),
        "bvrow": bu.reshape(1, C).astype(npbf),
        "ident": np.eye(P, dtype=np.float32).astype(npbf),
    }
    in_maps = []
    for i in range(R):
        m = dict(shared)
        xs = x[i * NL:(i + 1) * NL, :]  # [NL, C]
        m["xT"] = np.ascontiguousarray(
            xs.T.reshape(CT, P, NL).transpose(1, 0, 2).reshape(P, CT * NL)
        ).astype(npbf)
        in_maps.append(m)

    res = run_bass_kernel_spmd(nc, in_maps, core_ids=list(range(R)),
                               trace=TRACE)
    _CACHE["last_result"] = res

    out = np.empty((N, C), dtype=np.float32)
    for i in range(R):
        out[i * NL:(i + 1) * NL, :] = \
            res.results[i]["outT"].T.astype(np.float32)
    return out
